# revision 17
# baseline (speedup 1.0000x reference)
"""Trainium2 Bass kernel for EulerProductMoE (dense 6-expert MoE with 2x3 product gate).

Data-parallel over 8 NeuronCores: batch dim sharded (4096 tokens/core), all
weights replicated. Host does layout-only prep (fp16 cast + SBUF-ready
transposed packing); per core a single Tile program:
  - preloads W1/W2 (fp16) resident in SBUF via 3+3 large HWDGE DMAs
  - per 512-token chunk: DMA xT (feature-major, prepacked), product gate
    on-chip, layer1 (fp16 matmul, f32 accum, relu+bias, gate-scale),
    layer2 (N=1024 moving, + w6 @ b2 when b2 != 0), DMA out f32.
"""

import os
import sys

for _p in ("/opt/trn_rl_repo", "/root/.axon_site/_ro/trn_rl_repo"):
    if os.path.isdir(_p) and _p not in sys.path:
        sys.path.insert(0, _p)
        break

import numpy as np

import concourse.bass as bass  # noqa: E402
import concourse.mybir as mybir  # noqa: E402
import concourse.tile as tile  # noqa: E402
from concourse import bacc  # noqa: E402
from concourse.bass_utils import run_bass_kernel_spmd  # noqa: E402

F32 = mybir.dt.float32
F16 = mybir.dt.float16
AF = mybir.ActivationFunctionType
AX = mybir.AxisListType

N_CORES = 8
B_FULL = 32768
B_CORE = B_FULL // N_CORES  # 4096
IN_DIM = 1024
HID = 512
OUT_DIM = 1024
NE = 6
HID_CAT = NE * HID  # 3072
P = 128

CHUNK = 512  # tokens per chunk
NCHUNK = B_CORE // CHUNK  # 8
TT = CHUNK // P  # 4 token tiles per chunk
K1 = IN_DIM // P  # 8 contraction tiles for layer 1
M1 = HID_CAT // P  # 24 hid tiles
K2 = HID_CAT // P  # 24 contraction tiles for layer 2


def _build_kernel(ctx, tc, aps, skip_b2=False):
    nc = tc.nc
    xT, w1h, w2h, g4h, b1r, b2, g2_b, g3_b, out = aps

    wts = ctx.enter_context(tc.tile_pool(name="wts", bufs=1))
    xTp = ctx.enter_context(tc.tile_pool(name="xT", bufs=3))
    hTp = ctx.enter_context(tc.tile_pool(name="hT", bufs=1))
    wbcp = ctx.enter_context(tc.tile_pool(name="wbc", bufs=2))
    osbp = ctx.enter_context(tc.tile_pool(name="osb", bufs=3))
    gatep = ctx.enter_context(tc.tile_pool(name="gate", bufs=2))
    pl1 = ctx.enter_context(tc.tile_pool(name="pl1", bufs=2, space="PSUM"))
    pl2 = ctx.enter_context(tc.tile_pool(name="pl2", bufs=2, space="PSUM"))
    pg = ctx.enter_context(tc.tile_pool(name="pg", bufs=2, space="PSUM"))

    # ---- resident weights / constants ----
    W1sb = wts.tile([P, M1, K1, P], F16)  # [p, m, k, j]
    W2sb = wts.tile([P, K2, OUT_DIM], F16)  # [p, kh, o]
    b1sb = wts.tile([P, M1], F32)
    b2sb = wts.tile([NE, OUT_DIM], F16)
    gWsb = wts.tile([P, K1, 4], F16)  # gate weights (moving operand, N=4)
    g2bb = wts.tile([P, 1], F32)
    g3bb = wts.tile([P, 3], F32)
    idf32 = wts.tile([P, P], F32)
    w6Tfull = wts.tile([P, CHUNK], F16)

    def _bcast(ap, n):
        return bass.AP(tensor=ap.tensor, offset=ap.offset, ap=[[0, n], *ap.ap])

    # HAM warm-up fodder: memset first so the junk matmuls below only wait
    # on this one fast vector op
    junk = gatep.tile([P, CHUNK], F16, tag="junk")
    nc.vector.memset(junk[:], 0.0)

    # x chunk DMAs: feature-major prepacked on host; one 1MB DMA per chunk.
    # All on the sync HWDGE queue (fast start, FIFO with the weight loads).
    def emit_xt_dma(c):
        xt = xTp.tile([P, K1, CHUNK], F16)
        nc.sync.dma_start(out=xt[:], in_=xT[:, :, c * CHUNK : (c + 1) * CHUNK])
        return xt

    xt_tiles = [None] * NCHUNK
    xt_tiles[0] = emit_xt_dma(0)

    nc.sync.dma_start(out=gWsb[:], in_=g4h[:])
    nc.sync.dma_start(out=b1sb[:], in_=b1r[:])
    nc.sync.dma_start(out=g2bb[:], in_=_bcast(g2_b, P))
    nc.sync.dma_start(out=g3bb[:], in_=_bcast(g3_b, P))
    # identity as a NEFF-embedded constant
    nc.sync.dma_start(
        out=idf32[:],
        in_=nc.inline_tensor(np.eye(P).astype(np.float32), name="id32").ap(),
    )
    # only rows 0:6 of w6Tfull are used (b2 matmul lhsT); zero the rest once
    nc.vector.memset(w6Tfull[:], 0.0)
    nc.gpsimd.dma_start(out=b2sb[:], in_=b2[:])

    # resident weights on the sync HWDGE queue, ordered to match consumption:
    # W1 in 1MB pieces so chunk 0's first L1 m-tiles aren't gated on a big
    # transfer, xt chunk 1 after the first two W1 pieces, W2 kh-tiles last
    for i in range(2):
        nc.sync.dma_start(
            out=W1sb[:, i * 4 : (i + 1) * 4], in_=w1h[:, i * 4 : (i + 1) * 4]
        )
    xt_tiles[1] = emit_xt_dma(1)
    for i in range(2, 6):
        nc.sync.dma_start(
            out=W1sb[:, i * 4 : (i + 1) * 4], in_=w1h[:, i * 4 : (i + 1) * 4]
        )
    for i in range(3):
        nc.sync.dma_start(
            out=W2sb[:, i * 8 : (i + 1) * 8, :], in_=w2h[:, i * 8 : (i + 1) * 8, :]
        )

    # HAM warm-up: junk matmuls fill the ~3.4us cold-clock window while the
    # PE waits for the first x/gate DMAs; the PSUM result is never read
    def emit_junk(n):
        for i in range(n):
            pj = pg.tile([P, CHUNK], F32, tag="g")
            nc.tensor.matmul(pj[:], junk[:, :P], junk[:], start=True, stop=True)

    emit_junk(14)

    def emit_logits(xt):
        # x token-tiles stationary, the 4 gate columns moving (N=4): logits
        # come out token-major, no transposes needed
        lgt = pg.tile([P, TT, 4], F32, tag="g")
        for t in range(TT):
            for k in range(K1):
                nc.tensor.matmul(
                    lgt[:, t, :],
                    xt[:, k, t * P : (t + 1) * P],
                    gWsb[:, k, :],
                    start=(k == 0),
                    stop=(k == K1 - 1),
                )
        return lgt

    lg_cur = emit_logits(xt_tiles[0])

    for c in range(NCHUNK):
        tok0 = c * CHUNK
        xt = xt_tiles[c]

        # layer-1 helpers; gate/broadcast PE work is interleaved into the m
        # loop so the serialized gate chain hides behind L1 matmuls
        hT = hTp.tile([P, M1, CHUNK], F16)

        def emit_l1(m, hT=hT, xt=xt):
            ps = pl1.tile([P, CHUNK], F32)
            for k in range(K1):
                nc.tensor.matmul(
                    ps[:],
                    W1sb[:, m, k, :],
                    xt[:, k, :],
                    start=(k == 0),
                    stop=(k == K1 - 1),
                )
            nc.scalar.activation(hT[:, m, :], ps[:], AF.Relu, bias=b1sb[:, m : m + 1])

        # logits psum -> sbuf, already token-major
        lt = gatep.tile([P, TT, 4], F32, tag="lt")
        nc.vector.tensor_copy(lt[:], lg_cur[:])
        ltv = lt

        if c == 0:
            emit_junk(3)
        for m in range(0, 2):
            emit_l1(m)
        if c == 0:
            emit_junk(4)
        for m in range(2, 4):
            emit_l1(m)
        if c == 0:
            emit_junk(4)

        # prefetch the x tile two chunks ahead (reuses the c-th buffer after
        # this chunk's L1 finishes reading it)
        if c + 2 < NCHUNK and xt_tiles[c + 2] is None:
            xt_tiles[c + 2] = emit_xt_dma(c + 2)

        # batched gate math: one Exp for all 16 logits; sigmoid via reciprocal
        ge = gatep.tile([P, TT, 4], F32, tag="ge")
        # ge[.,t,0] = -(l2 + g2b); ge[.,t,1:4] = l3 + g3b
        nc.vector.tensor_scalar(
            ge[:, :, 0],
            ltv[:, :, 0],
            g2bb[:, 0:1],
            -1.0,
            mybir.AluOpType.add,
            mybir.AluOpType.mult,
        )
        for t in range(TT):
            nc.vector.tensor_add(ge[:, t, 1:4], ltv[:, t, 1:4], g3bb[:])
        nc.scalar.activation(ge[:], ge[:], AF.Exp)
        a1 = gatep.tile([P, TT], F32, tag="a1")
        nc.vector.tensor_scalar_add(a1[:], ge[:, :, 0], 1.0)
        sig = gatep.tile([P, TT], F32, tag="sig")
        nc.vector.reciprocal(sig[:], a1[:])  # sigmoid(l2+g2b)
        dn = gatep.tile([P, TT, 1], F32, tag="dn")
        nc.vector.reduce_sum(dn[:], ge[:, :, 1:4], axis=AX.X)
        rdn = gatep.tile([P, TT, 1], F32, tag="rdn")
        nc.vector.reciprocal(rdn[:], dn[:])
        A1 = gatep.tile([P, TT], F32, tag="A1")  # a/denom
        nc.vector.tensor_mul(A1[:], sig[:], rdn[:, :, 0])
        A0 = gatep.tile([P, TT], F32, tag="A0")  # (1-a)/denom
        nc.vector.tensor_sub(A0[:], rdn[:, :, 0], A1[:])
        w6 = gatep.tile([P, NE * TT], F32, tag="w6")
        for t in range(TT):
            nc.vector.tensor_scalar_mul(
                w6[:, t * 6 : t * 6 + 3], ge[:, t, 1:4], A0[:, t : t + 1]
            )
            nc.vector.tensor_scalar_mul(
                w6[:, t * 6 + 3 : t * 6 + 6], ge[:, t, 1:4], A1[:, t : t + 1]
            )

        for m in range(4, 8):
            emit_l1(m)

        # w6 -> expert-major [6, tok] (PE transposes, interleaved with L1)
        w6Tp = pg.tile([NE, CHUNK], F32, tag="g")
        for t in range(TT):
            nc.tensor.transpose(
                w6Tp[:, t * P : (t + 1) * P], w6[:, t * 6 : (t + 1) * 6], idf32[:]
            )
        nc.scalar.copy(w6Tfull[:NE, :], w6Tp[:])
        for m in range(8, 10):
            emit_l1(m)

        # broadcast w6T rows to 128 partitions on the (idle) gpsimd engine;
        # emit each hT gate-scale as soon as its relu-evict and wbc row exist
        def emit_scale(m, hT=hT):
            nc.vector.tensor_mul(hT[:, m, :], hT[:, m, :], wbc[:, m // 4, :])

        wbc = wbcp.tile([P, NE, CHUNK], F16)
        w6flat = gatep.tile([1, NE * CHUNK], F16, tag="w6f")
        nc.gpsimd.dma_start(out=w6flat[:], in_=w6Tfull[:NE, :])
        nc.gpsimd.partition_broadcast(wbc[:], w6flat[:])
        scale_plan = {10: [0, 1, 2, 3], 11: [4, 5, 6, 7], 12: [8, 9, 10, 11],
                      13: [12, 13], 14: [14], 15: [15]}
        for m in range(10, 16):
            emit_l1(m)
            for mp in scale_plan[m]:
                emit_scale(mp)
        for m in range(16, M1):
            emit_l1(m)
            emit_scale(m)

        # stage the next chunk's gate logits ahead of this chunk's L2 so the
        # chunk boundary has no PE bubble
        if c + 1 < NCHUNK:
            lg_cur = emit_logits(xt_tiles[c + 1])

        # ---- layer 2: out[tok, o] = hT_w.T @ W2 (+ w6.T @ b2) ----
        for t in range(TT):
            osb = osbp.tile([P, OUT_DIM], F32)
            for n in range(2):
                ps2 = pl2.tile([P, 512], F32)
                for kh in range(K2):
                    nc.tensor.matmul(
                        ps2[:],
                        hT[:, kh, t * P : (t + 1) * P],
                        W2sb[:, kh, n * 512 : (n + 1) * 512],
                        start=(kh == 0),
                        stop=(skip_b2 and kh == K2 - 1),
                    )
                if not skip_b2:
                    nc.tensor.matmul(
                        ps2[:],
                        w6Tfull[:NE, t * P : (t + 1) * P],
                        b2sb[:, n * 512 : (n + 1) * 512],
                        start=False,
                        stop=True,
                    )
                nc.vector.tensor_copy(osb[:, n * 512 : (n + 1) * 512], ps2[:])
                nc.scalar.dma_start(
                    out=out[
                        tok0 + t * P : tok0 + (t + 1) * P, n * 512 : (n + 1) * 512
                    ],
                    in_=osb[:, n * 512 : (n + 1) * 512],
                )


def build(skip_b2=False):
    nc = bacc.Bacc("TRN2", target_bir_lowering=False, debug=False)
    aps = (
        nc.dram_tensor("xT", [P, K1, B_CORE], F16, kind="ExternalInput").ap(),
        nc.dram_tensor("w1h", [P, M1, K1, P], F16, kind="ExternalInput").ap(),
        nc.dram_tensor("w2h", [P, K2, OUT_DIM], F16, kind="ExternalInput").ap(),
        nc.dram_tensor("g4h", [P, K1, 4], F16, kind="ExternalInput").ap(),
        nc.dram_tensor("b1r", [P, M1], F32, kind="ExternalInput").ap(),
        nc.dram_tensor("b2", [NE, OUT_DIM], F32, kind="ExternalInput").ap(),
        nc.dram_tensor("g2_b", [1], F32, kind="ExternalInput").ap(),
        nc.dram_tensor("g3_b", [3], F32, kind="ExternalInput").ap(),
        nc.dram_tensor("out", [B_CORE, OUT_DIM], F32, kind="ExternalOutput").ap(),
    )
    from contextlib import ExitStack

    with tile.TileContext(nc) as tc, ExitStack() as ctx:
        _build_kernel(ctx, tc, aps, skip_b2=skip_b2)
    nc.compile()
    return nc


_NC_CACHE = {}


def _get_nc(skip_b2=False):
    if skip_b2 not in _NC_CACHE:
        _NC_CACHE[skip_b2] = build(skip_b2=skip_b2)
    return _NC_CACHE[skip_b2]


def _prep_shared(inputs):
    f32 = np.float32
    f16 = np.float16
    w1 = np.asarray(inputs["w1"], dtype=f32)
    w2 = np.asarray(inputs["w2"], dtype=f32)
    # W1cat[f, e*HID + h] = w1[e, f, h]; repack to [p, m, k, j] with
    # f = k*128 + p, col = m*128 + j
    w1cat = w1.transpose(1, 0, 2).reshape(IN_DIM, HID_CAT)
    w1h = np.ascontiguousarray(
        w1cat.reshape(K1, P, M1, P).transpose(1, 2, 0, 3).astype(f16)
    )
    # W2cat[e*HID + hh, o] = w2[e, hh, o]; repack to [p, kh, o]
    w2h = np.ascontiguousarray(
        w2.reshape(HID_CAT, OUT_DIM).reshape(K2, P, OUT_DIM).transpose(1, 0, 2).astype(f16)
    )
    g4 = np.concatenate(
        [np.asarray(inputs["g2_w"], f32), np.asarray(inputs["g3_w"], f32)], axis=1
    )  # [IN_DIM, 4]
    g4h = np.ascontiguousarray(g4.reshape(K1, P, 4).transpose(1, 0, 2).astype(f16))
    b1r = np.ascontiguousarray(
        np.asarray(inputs["b1"], f32).reshape(HID_CAT).reshape(M1, P).T
    )
    return {
        "w1h": w1h,
        "w2h": w2h,
        "g4h": g4h,
        "b1r": b1r,
        "b2": np.ascontiguousarray(inputs["b2"], dtype=f32),
        "g2_b": np.ascontiguousarray(inputs["g2_b"], dtype=f32),
        "g3_b": np.ascontiguousarray(inputs["g3_b"], dtype=f32),
    }


def _in_maps(inputs):
    x16 = np.asarray(inputs["x"], dtype=np.float32).astype(np.float16)
    shared = _prep_shared(inputs)
    maps = []
    for i in range(N_CORES):
        xc = x16[i * B_CORE : (i + 1) * B_CORE]  # [B_CORE, IN_DIM]
        xTh = np.ascontiguousarray(
            xc.T.reshape(K1, P, B_CORE).transpose(1, 0, 2)
        )  # [P, K1, B_CORE]
        maps.append({"xT": xTh, **shared})
    return maps


def run(inputs, **kw):
    skip_b2 = bool(np.all(np.asarray(inputs["b2"]) == 0))
    nc = _get_nc(skip_b2=skip_b2)
    res = run_bass_kernel_spmd(nc, _in_maps(inputs), list(range(N_CORES)), **kw)
    full = np.concatenate([res.results[i]["out"] for i in range(N_CORES)], axis=0)
    return full, res


def kernel(**inputs) -> np.ndarray:
    full, _ = run(inputs)
    return full
